# revision 1
# baseline (speedup 1.0000x reference)
"""CRF loss kernel for Trainium2 (8 NeuronCores, data-parallel over batch).

Problem (hardcoded shapes): scores [B=128, T=256, K=64, K=64] f32,
targets [128, 256] int (flattened from_tag*K + to_tag), lengths [128] int.

loss = (sum_b fs[b, END] - gold) / B  where fs is the CRF forward
(log-domain) scan and gold is the gathered gold-path score.

Strategy (per core, 16 batch rows):
  * Forward scan in the *linear* domain with a constant per-step scale
    1/C (C = 128): a_t = C^-t * alpha_t, where
    alpha_t[kto] = sum_kf exp(sc_t[kf, kto]) * alpha_{t-1}[kf].
    exp(sc) is computed by ScalarE on [128, K*W] strips (off the serial
    critical path); the kf-contraction runs on TensorE as 8 pair-stacked
    matmuls ([E_b0; E_b1] [128,64] weights x staggered [128,2] rhs).
  * The staggered rhs_t ([128, 8]: col j holds a_t
    for row 2j in partitions 0-63 and row 2j+1 in partitions 64-127,
    zeros elsewhere) is dumped to DRAM every step; the host reads the
    frozen state a_{L_b-1} per row and finishes with log + offsets.
    This keeps the compiled kernel identical across cores (SPMD) with
    no data-dependent control flow on device.
  * gold: indirect DMA element-gather of scores[b,t,kf*,kto*] for all
    (b,t), masked by validity (t < L_b) and reduced on device.
"""

import math

import numpy as np

import concourse.bacc as bacc
import concourse.bass as bass
import concourse.tile as tile
from concourse import mybir
from concourse.bass_utils import run_bass_kernel_spmd

F32 = mybir.dt.float32
I32 = mybir.dt.int32

B = 128
T = 256
K = 64
START = 62
END = 63
NCORES = 8
BL = B // NCORES          # 16 local batch rows per core
NPAIR = BL // 2           # 8
W = 16                    # timesteps per DMA/exp strip block
G = BL * T // 128         # gold gather indices per partition (32)
C_SCALE = 1.0 / 128.0     # per-step normalizer 1/C
LOG_C = 7.0 * math.log(2.0)  # log(128)
NEG_BIG = -1.0e30         # exp(NEG_BIG) == 0 filler


def _build_nc():
    nc = bacc.Bacc("TRN2", target_bir_lowering=False)

    sc = nc.dram_tensor("scores", [BL, T, K, K], F32, kind="ExternalInput")
    gidx = nc.dram_tensor("gidx", [128, G], I32, kind="ExternalInput")
    states = nc.dram_tensor("states", [T, 128, BL], F32, kind="ExternalOutput")
    gold = nc.dram_tensor("gold", [1, 1], F32, kind="ExternalOutput")

    with tile.TileContext(nc) as tc:
        with (
            tc.tile_pool(name="strips", bufs=2) as strips,
            tc.tile_pool(name="persist", bufs=1) as persist,
            tc.tile_pool(name="pers_psum", bufs=1, space="PSUM") as pers_psum,
        ):
            # ---- persistent tiles -------------------------------------
            rhs_bufs = [
                persist.tile([128, BL], F32, tag=f"rhs{i}", name=f"rhs{i}")
                for i in range(3)
            ]
            psum_bufs = [
                pers_psum.tile([K, BL], F32, tag=f"ps{i}", name=f"ps{i}")
                for i in range(2)
            ]

            # ---- gold gather (runs concurrently with the scan) --------
            # invalid (padded) positions carry a huge sentinel index; the
            # bounds check silently skips them, leaving the pre-zeroed
            # elements untouched, so no mask pairing is needed and the
            # final sum is independent of the gather's output layout.
            idxs = persist.tile([128, G], I32, tag="idxs", name="idxs")
            gath = persist.tile([128, G], F32, tag="gath", name="gath")
            nc.gpsimd.dma_start(out=idxs[:], in_=gidx[:])
            nc.gpsimd.memset(gath[:], 0.0)
            sc_flat = sc[:].rearrange(
                "b t kf (kto one) -> (b t kf kto) one", one=1
            )
            nc.gpsimd.indirect_dma_start(
                out=gath[:],
                out_offset=None,
                in_=sc_flat,
                in_offset=bass.IndirectOffsetOnAxis(ap=idxs[:], axis=0),
                bounds_check=BL * T * K * K - 1,
                oob_is_err=False,
            )

            # ---- init: a_0 = exp(scores[b, 0, START, :]) --------------
            # single DMA into a dense staging tile, exp there, then two
            # stagger copies into a zeroed rhs0 (keeps every instruction's
            # sync-wait count tiny).
            staging = persist.tile([K, BL], F32, tag="staging", name="staging")
            nc.sync.dma_start(
                out=staging[:],
                in_=sc[:, 0, START, :].rearrange("b k -> k b"),
            )
            nc.scalar.activation(
                staging[:], staging[:], mybir.ActivationFunctionType.Exp
            )
            rhs0 = rhs_bufs[0]
            nc.vector.memset(rhs0[:], 0.0)
            nc.vector.memset(rhs_bufs[1][:], 0.0)
            nc.vector.memset(rhs_bufs[2][:], 0.0)
            nc.vector.tensor_copy(rhs0[0:64, 0:BL:2], staging[:, 0:BL:2])
            nc.vector.tensor_copy(rhs0[64:128, 1:BL:2], staging[:, 1:BL:2])
            nc.sync.dma_start(out=states[0], in_=rhs0[:])

            # ---- main scan --------------------------------------------
            rhs_prev = rhs0
            for blk in range(T // W):
                cur = []
                for j in range(NPAIR):
                    s = strips.tile([128, W * K], F32, tag=f"strip{j}")
                    for h in range(2):
                        b = 2 * j + h
                        nc.sync.dma_start(
                            out=s[64 * h : 64 * h + 64, :].rearrange(
                                "p (t k) -> p t k", t=W
                            ),
                            in_=sc[b, blk * W : (blk + 1) * W].rearrange(
                                "t kf kto -> kf t kto"
                            ),
                        )
                    nc.scalar.activation(
                        s[:], s[:], mybir.ActivationFunctionType.Exp
                    )
                    cur.append(s)

                for tl in range(W):
                    t = blk * W + tl
                    if t == 0:
                        continue
                    ps = psum_bufs[t % 2]
                    rhs_new = rhs_bufs[t % 3]
                    for j in range(NPAIR):
                        nc.tensor.matmul(
                            out=ps[:, 2 * j : 2 * j + 2],
                            lhsT=cur[j][:, tl * K : (tl + 1) * K],
                            rhs=rhs_prev[:, 2 * j : 2 * j + 2],
                            start=True,
                            stop=True,
                        )
                    # staggered copy psum -> next rhs, with 1/C scaling
                    nc.vector.tensor_scalar_mul(
                        rhs_new[0:64, 0 : BL : 2], ps[0:64, 0 : BL : 2], C_SCALE
                    )
                    nc.vector.tensor_scalar_mul(
                        rhs_new[64:128, 1 : BL : 2], ps[0:64, 1 : BL : 2], C_SCALE
                    )
                    nc.sync.dma_start(out=states[t], in_=rhs_new[:])
                    rhs_prev = rhs_new

            # ---- gold reduce (entirely on GPSIMD, off the scan path) --
            goldsb = persist.tile([1, 1], F32, tag="goldsb", name="goldsb")
            nc.gpsimd.tensor_reduce(
                goldsb[:],
                gath[:],
                axis=mybir.AxisListType.XYZWC,
                op=mybir.AluOpType.add,
            )
            nc.gpsimd.dma_start(out=gold[:], in_=goldsb[:])

    return nc


_NC_CACHE = None


def _get_nc():
    global _NC_CACHE
    if _NC_CACHE is None:
        _NC_CACHE = _build_nc()
        _NC_CACHE.finalize()
    return _NC_CACHE


def _make_in_maps(scores, targets, lengths):
    scores = np.ascontiguousarray(np.asarray(scores, dtype=np.float32))
    targets = np.asarray(targets).astype(np.int64)
    lengths = np.asarray(lengths).astype(np.int64)

    in_maps = []
    for c in range(NCORES):
        sl = slice(c * BL, (c + 1) * BL)
        sc_shard = scores[sl]
        tg = targets[sl]          # [BL, T]
        ln = lengths[sl]          # [BL]

        # element index into flattened local scores [BL*T*K*K]
        b_idx = np.arange(BL)[:, None]
        t_idx = np.arange(T)[None, :]
        flat = (b_idx * T + t_idx) * (K * K) + tg  # [BL, T]
        valid = t_idx < ln[:, None]  # [BL, T]
        flat = np.where(valid, flat, np.int64(0x7FFFFF00))
        gidx = flat.reshape(128, G).astype(np.int32)

        in_maps.append(
            {
                "scores": sc_shard,
                "gidx": np.ascontiguousarray(gidx),
            }
        )
    return in_maps, lengths


def _combine(results, lengths):
    all_scores = 0.0
    gold_total = 0.0
    for c in range(NCORES):
        st = results[c]["states"]  # [T, 128, BL]
        gold_total += float(results[c]["gold"][0, 0])
        for bl in range(BL):
            L = int(lengths[c * BL + bl])
            tau = L - 1
            a_end = float(st[tau, (bl % 2) * 64 + END, bl])
            all_scores += math.log(a_end) + tau * LOG_C
    return np.float32((all_scores - gold_total) / B)


def kernel(scores, targets, lengths, trace=False):
    nc = _get_nc()
    in_maps, ln = _make_in_maps(scores, targets, lengths)
    res = run_bass_kernel_spmd(
        nc, in_maps, core_ids=list(range(NCORES)), trace=trace
    )
    out = _combine(res.results, ln)
    if trace:
        return out, res
    return out



# revision 6
# speedup vs baseline: 2.0995x; 2.0995x over previous
"""CRF loss kernel for Trainium2 (8 NeuronCores, data-parallel over batch).

Problem (hardcoded shapes): scores [B=128, T=256, K=64, K=64] f32,
targets [128, 256] int (flattened from_tag*K + to_tag), lengths [128] int.

loss = (sum_b fs[b, END] - gold) / B  where fs is the CRF forward
(log-domain) scan and gold is the gathered gold-path score.

Strategy (per core, 16 batch rows):
  * Linear-domain scan with the per-step 1/C normalizer (C = 128)
    folded into the exp: E'_t = exp(sc_t - log C), a_t = E'_t^T a_{t-1},
    so log alpha_tau = log a_tau + tau*log C.
  * exp is computed by ScalarE as f32 -> bf16 strips; the kf-contraction
    runs on TensorE as 4 "duo" matmuls per step: lhsT [128, 128] bf16
    packs FOUR batch rows (two stacked row-pairs side by side in the
    column dim), rhs [128, 4] holds the staggered previous state.
  * The full state history a_t lives in SBUF ([128, T*16] bf16,
    col 16*t + r = row r's state, even rows on partitions 0-63, odd on
    64-127) and is dumped to DRAM once at the end; the host reads the
    frozen state a_{L_b-1} per row and finishes with log + offsets.
  * gold: indirect DMA element-gather of scores[b,t,kf*,kto*] for all
    (b,t), invalid positions skipped via an OOB sentinel index, reduced
    along the free dim on DVE and summed on host.
  * Strip DMAs are split across both HWDGE rings (sync + scalar
    engines) to double DMA queue parallelism.
"""

import math

import numpy as np

import concourse.bacc as bacc
import concourse.bass as bass
import concourse.tile as tile
from concourse import mybir
from concourse.bass_utils import run_bass_kernel_spmd

F32 = mybir.dt.float32
BF16 = mybir.dt.bfloat16
I32 = mybir.dt.int32

B = 128
T = 256
K = 64
START = 62
END = 63
NCORES = 8
BL = B // NCORES          # 16 local batch rows per core
NPAIR = BL // 2           # 8 row-pairs
NDUO = BL // 4            # 4 duos (2 pairs each)
W = 16                    # timesteps per DMA/exp strip block
G = BL * T // 128         # gold gather indices per partition (32)
LOG_C = 7.0 * math.log(2.0)  # log(128); E' = exp(sc - LOG_C)


def _build_nc():
    nc = bacc.Bacc("TRN2", target_bir_lowering=False)

    sc = nc.dram_tensor("scores", [BL, T, K, K], F32, kind="ExternalInput")
    gidx = nc.dram_tensor("gidx", [128, G], I32, kind="ExternalInput")
    states = nc.dram_tensor("states", [128, T * BL], BF16, kind="ExternalOutput")
    gold = nc.dram_tensor("gold", [128, 1], F32, kind="ExternalOutput")

    with tile.TileContext(nc) as tc:
        with (
            tc.tile_pool(name="strips", bufs=3) as strips,
            tc.tile_pool(name="duos", bufs=2) as duos,
            tc.tile_pool(name="persist", bufs=1) as persist,
            tc.tile_pool(name="pers_psum", bufs=1, space="PSUM") as pers_psum,
        ):
            # ---- persistent state history -----------------------------
            st = persist.tile([128, T * BL], BF16, tag="st", name="st")
            nc.vector.memset(st[:], 0.0)

            # per-partition bias feeding exp(sc - log C)
            biasc = persist.tile([128, 1], F32, tag="biasc", name="biasc")
            nc.vector.memset(biasc[:], -LOG_C)

            psum_bufs = [
                pers_psum.tile([128, BL], F32, tag=f"ps{i}", name=f"ps{i}")
                for i in range(2)
            ]

            # ---- gold gather (runs concurrently with the scan) --------
            # invalid (padded) positions carry a huge sentinel index; the
            # bounds check silently skips them, leaving the pre-zeroed
            # elements untouched, so the free-dim reduce is exact.
            idxs = persist.tile([128, G], I32, tag="idxs", name="idxs")
            gath = persist.tile([128, G], F32, tag="gath", name="gath")
            goldsb = persist.tile([128, 1], F32, tag="goldsb", name="goldsb")
            nc.gpsimd.dma_start(out=idxs[:], in_=gidx[:])
            nc.gpsimd.memset(gath[:], 0.0)
            sc_flat = sc[:].rearrange(
                "b t kf (kto one) -> (b t kf kto) one", one=1
            )
            nc.gpsimd.indirect_dma_start(
                out=gath[:],
                out_offset=None,
                in_=sc_flat,
                in_offset=bass.IndirectOffsetOnAxis(ap=idxs[:], axis=0),
                bounds_check=BL * T * K * K - 1,
                oob_is_err=False,
            )
            nc.vector.tensor_reduce(
                goldsb[:],
                gath[:],
                axis=mybir.AxisListType.XYZW,
                op=mybir.AluOpType.add,
            )
            nc.sync.dma_start(out=gold[:], in_=goldsb[:])

            # ---- init: a_0 = exp(scores[b, 0, START, :]) --------------
            staging = persist.tile([K, BL], F32, tag="staging", name="staging")
            stg16 = persist.tile([K, BL], BF16, tag="stg16", name="stg16")
            nc.sync.dma_start(
                out=staging[:],
                in_=sc[:, 0, START, :].rearrange("b k -> k b"),
            )
            nc.scalar.activation(
                stg16[:], staging[:], mybir.ActivationFunctionType.Exp
            )
            nc.vector.tensor_copy(st[0:64, 0:BL:2], stg16[:, 0:BL:2])
            nc.vector.tensor_copy(st[64:128, 1:BL:2], stg16[:, 1:BL:2])

            # ---- main scan --------------------------------------------
            for blk in range(T // W):
                # load 8 pair-strips (split across the two HWDGE rings),
                # exp each into its duo lhsT tile (bf16, 1/C folded in).
                cur = []
                for j in range(NPAIR):
                    s = strips.tile([128, W * K], F32, tag=f"strip{j}")
                    eng = nc.sync if j % 2 == 0 else nc.scalar
                    for h in range(2):
                        eng.dma_start(
                            out=s[64 * h : 64 * h + 64, :].rearrange(
                                "p (t k) -> p t k", t=W
                            ),
                            in_=sc[2 * j + h, blk * W : (blk + 1) * W]
                            .rearrange("t kf kto -> kf t kto"),
                        )
                    cur.append(s)
                dtiles = []
                for d in range(NDUO):
                    dt = duos.tile([128, W * 2 * K], BF16, tag=f"duo{d}")
                    dv = dt[:].rearrange("p (t two k) -> p t two k", two=2, k=K)
                    for h in range(2):
                        j = 2 * d + h
                        nc.scalar.activation(
                            dv[:, :, h, :],
                            cur[j][:].rearrange("p (t k) -> p t k", t=W),
                            mybir.ActivationFunctionType.Exp,
                            bias=biasc[:],
                        )
                    dtiles.append(dt)

                for tl in range(W):
                    t = blk * W + tl
                    if t == 0:
                        continue
                    ps = psum_bufs[t % 2]
                    prev = st[:, (t - 1) * BL : t * BL]
                    for d in range(NDUO):
                        nc.tensor.matmul(
                            out=ps[:, 4 * d : 4 * d + 4],
                            lhsT=dtiles[d][:, tl * 2 * K : (tl + 1) * 2 * K],
                            rhs=prev[:, 4 * d : 4 * d + 4],
                            start=True,
                            stop=True,
                        )
                    # stagger psum -> state cols 16t..16t+15 (bf16 cast):
                    # col 4d+0: pair-A even row (psum top -> top)
                    # col 4d+1: pair-A odd row  (psum top -> bottom)
                    # col 4d+2: pair-B even row (psum bottom -> top)
                    # col 4d+3: pair-B odd row  (psum bottom -> bottom)
                    new = st[:, t * BL : (t + 1) * BL]
                    nc.vector.tensor_copy(new[0:64, 0:BL:4], ps[0:64, 0:BL:4])
                    nc.vector.tensor_copy(new[64:128, 1:BL:4], ps[0:64, 1:BL:4])
                    nc.vector.tensor_copy(new[0:64, 2:BL:4], ps[64:128, 2:BL:4])
                    nc.vector.tensor_copy(new[64:128, 3:BL:4], ps[64:128, 3:BL:4])

            # ---- dump full state history ------------------------------
            nc.sync.dma_start(out=states[:], in_=st[:])

    return nc


_NC_CACHE = None


def _get_nc():
    global _NC_CACHE
    if _NC_CACHE is None:
        _NC_CACHE = _build_nc()
        _NC_CACHE.finalize()
    return _NC_CACHE


def _make_in_maps(scores, targets, lengths):
    scores = np.ascontiguousarray(np.asarray(scores, dtype=np.float32))
    targets = np.asarray(targets).astype(np.int64)
    lengths = np.asarray(lengths).astype(np.int64)

    in_maps = []
    for c in range(NCORES):
        sl = slice(c * BL, (c + 1) * BL)
        sc_shard = scores[sl]
        tg = targets[sl]          # [BL, T]
        ln = lengths[sl]          # [BL]

        # element index into flattened local scores [BL*T*K*K]
        b_idx = np.arange(BL)[:, None]
        t_idx = np.arange(T)[None, :]
        flat = (b_idx * T + t_idx) * (K * K) + tg  # [BL, T]
        valid = t_idx < ln[:, None]  # [BL, T]
        flat = np.where(valid, flat, np.int64(0x7FFFFF00))
        gidx = flat.reshape(128, G).astype(np.int32)

        in_maps.append(
            {
                "scores": sc_shard,
                "gidx": np.ascontiguousarray(gidx),
            }
        )
    return in_maps, lengths


def _combine(results, lengths):
    all_scores = 0.0
    gold_total = 0.0
    for c in range(NCORES):
        stv = np.asarray(results[c]["states"], dtype=np.float32)  # [128, T*BL]
        gold_total += float(
            np.asarray(results[c]["gold"], dtype=np.float32).sum()
        )
        for bl in range(BL):
            L = int(lengths[c * BL + bl])
            tau = L - 1
            a_end = float(stv[(bl % 2) * 64 + END, tau * BL + bl])
            all_scores += math.log(a_end) + tau * LOG_C
    return np.float32((all_scores - gold_total) / B)


def kernel(scores, targets, lengths, trace=False):
    nc = _get_nc()
    in_maps, ln = _make_in_maps(scores, targets, lengths)
    res = run_bass_kernel_spmd(
        nc, in_maps, core_ids=list(range(NCORES)), trace=trace
    )
    out = _combine(res.results, ln)
    if trace:
        return out, res
    return out
